# revision 5
# baseline (speedup 1.0000x reference)
"""Multi-head attention (B=8, S=1024, D=1024, H=16) on 8 Trainium2 NeuronCores.

Sharding: data-parallel over batch — core b handles batch b end-to-end
(projections, attention, output projection). No collectives.

Per-core kernel (S=1024 tokens, D=1024, H=16 heads, HD=64):
  X^T via PE transpose  ->  Q^T/K^T (feature-major) and V (token-major)
  scores = (0.125*Q)^T K per head, rel_bias accumulated into PSUM via
  identity matmul, exp+rowsum on ScalarE, normalize on VectorE
  (the normalized tiles feed both the `attn` DRAM output and the
  PE-transposed E^T used for attn @ V), final projection from O^T.

All matmuls run in float32r (TF32-like, ~1e-4 relative error, full PE rate).
"""

import numpy as np

import concourse.bass as bass
import concourse.tile as tile
from concourse import bacc, masks, mybir

B, S, D, H = 8, 1024, 1024, 16
HD = D // H  # 64
T = S // 128  # 8 q/token tiles
NCORES = 8

F32 = mybir.dt.float32
F32R = mybir.dt.float32r
I32 = mybir.dt.int32
AF = mybir.ActivationFunctionType
ALU = mybir.AluOpType


def build_nc(handle_mask=False, has_bq=False, has_bk=False, has_bv=False, has_bo=False):
    nc = bacc.Bacc("TRN2", target_bir_lowering=False, debug=False, num_devices=NCORES)

    xq = nc.dram_tensor("xq", (S, D), F32, kind="ExternalInput").ap()
    xk = nc.dram_tensor("xk", (S, D), F32, kind="ExternalInput").ap()
    xv = nc.dram_tensor("xv", (S, D), F32, kind="ExternalInput").ap()
    wq = nc.dram_tensor("wq", (D, D), F32R, kind="ExternalInput").ap()
    wk = nc.dram_tensor("wk", (D, D), F32R, kind="ExternalInput").ap()
    wv = nc.dram_tensor("wv", (D, D), F32R, kind="ExternalInput").ap()
    wo = nc.dram_tensor("wo", (D, D), F32R, kind="ExternalInput").ap()
    rb = nc.dram_tensor("rb", (H, S, S), F32R, kind="ExternalInput").ap()
    mask = None
    if handle_mask:
        mask = nc.dram_tensor("mask", (S, S), I32, kind="ExternalInput").ap()
    bqt = bkt = bvt = bot = None
    if has_bq:
        bqt = nc.dram_tensor("bq", (T, 128), F32, kind="ExternalInput").ap()
    if has_bk:
        bkt = nc.dram_tensor("bk", (T, 128), F32, kind="ExternalInput").ap()
    if has_bv:
        bvt = nc.dram_tensor("bv", (1, D), F32R, kind="ExternalInput").ap()
    if has_bo:
        bot = nc.dram_tensor("bo", (1, D), F32R, kind="ExternalInput").ap()

    out = nc.dram_tensor("out", (S, D), F32, kind="ExternalOutput").ap()
    attn = nc.dram_tensor("attn", (H, S, S), F32, kind="ExternalOutput").ap()

    with tile.TileContext(nc) as tc:
        _emit(nc, tc, locals())
    nc.compile()
    return nc


def _emit(nc, tc, g):
    xq, xk, xv = g["xq"], g["xk"], g["xv"]
    wq, wk, wv, wo = g["wq"], g["wk"], g["wv"], g["wo"]
    rb, mask, out, attn = g["rb"], g["mask"], g["out"], g["attn"]
    bqt, bkt, bvt, bot = g["bqt"], g["bkt"], g["bvt"], g["bot"]

    from contextlib import ExitStack

    ctx = ExitStack()
    with ctx:
        const_pool = ctx.enter_context(tc.tile_pool(name="const", bufs=1))
        ident = const_pool.tile([128, 128], F32, tag="ident")
        masks.make_identity(nc, ident[:])
        ident_r = const_pool.tile([128, 128], F32R, tag="identr")
        nc.vector.tensor_copy(ident_r[:], ident[:])
        ones_r = None
        if bvt is not None or bot is not None:
            ones_r = const_pool.tile([1, 128], F32R, tag="ones")
            nc.gpsimd.memset(ones_r[:], 1.0)

        # persistent activations (live through attention / final phases)
        qt_pool = ctx.enter_context(tc.tile_pool(name="qt", bufs=T))
        kt_pool = ctx.enter_context(tc.tile_pool(name="kt", bufs=T))
        vt_pool = ctx.enter_context(tc.tile_pool(name="vt", bufs=T))

        # ---------------- Phase P: projections ----------------
        QT, KT, VT = [], [], []
        with (
            tc.tile_pool(name="xin", bufs=5) as xin_pool,
            tc.tile_pool(name="xt", bufs=T) as xt_pool,
            tc.tile_pool(name="wts", bufs=T) as w_pool,
            tc.tile_pool(name="bias", bufs=2) as b_pool,
            tc.tile_pool(name="pxt", bufs=2, space="PSUM") as pxt_pool,
            tc.tile_pool(name="ppj", bufs=4, space="PSUM") as ppj_pool,
        ):
            for x_dram, w_dram, kind, b_dram in (
                (xq, wq, "q", bqt),
                (xk, wk, "k", bkt),
                (xv, wv, "v", bvt),
            ):
                # X^T: PE-transpose X into feature-major f32r tiles
                xt = [xt_pool.tile([128, S], F32R, tag="xt", name=f"xt_{kind}_{j}") for j in range(T)]
                for half in range(2):
                    xin = []
                    for t in range(half * 4, half * 4 + 4):
                        xi = xin_pool.tile([128, D], F32, tag="xin")
                        nc.sync.dma_start(xi[:], x_dram[t * 128 : (t + 1) * 128, :])
                        xin.append(xi)
                    for j in range(T):
                        pt = pxt_pool.tile([128, 512], F32, tag="pxt")
                        for i in range(4):
                            nc.tensor.transpose(
                                pt[:, i * 128 : (i + 1) * 128],
                                xin[i][:, j * 128 : (j + 1) * 128],
                                ident[:],
                            )
                        nc.any.tensor_copy(
                            xt[j][:, half * 512 : (half + 1) * 512], pt[:]
                        )

                wt = []
                for k in range(T):
                    wtile = w_pool.tile([128, D], F32R, tag="w")
                    nc.sync.dma_start(wtile[:], w_dram[k * 128 : (k + 1) * 128, :])
                    wt.append(wtile)

                if kind in ("q", "k"):
                    # out = W^T X^T  (feature-major), fold 0.125 scale for q
                    btile = None
                    if b_dram is not None:
                        btile = b_pool.tile([128, T], F32, tag="pb")
                        nc.sync.dma_start(btile[:], b_dram.rearrange("m p -> p m"))
                    dst_list = QT if kind == "q" else KT
                    pool = qt_pool if kind == "q" else kt_pool
                    scale = 0.125 if kind == "q" else 1.0
                    for m in range(T):
                        dst = pool.tile([128, S], F32R, tag="t")
                        for n in range(2):
                            pp = ppj_pool.tile([128, 512], F32, tag="ppj")
                            for k in range(T):
                                nc.tensor.matmul(
                                    pp[:],
                                    wt[k][:, m * 128 : (m + 1) * 128],
                                    xt[k][:, n * 512 : (n + 1) * 512],
                                    start=(k == 0),
                                    stop=(k == T - 1),
                                )
                            seg = dst[:, n * 512 : (n + 1) * 512]
                            if btile is not None:
                                nc.vector.tensor_scalar(
                                    seg,
                                    pp[:],
                                    btile[:, m : m + 1],
                                    scale,
                                    op0=ALU.add,
                                    op1=ALU.mult,
                                )
                            elif scale != 1.0:
                                nc.vector.tensor_scalar_mul(seg, pp[:], scale)
                            else:
                                nc.any.tensor_copy(seg, pp[:])
                        dst_list.append(dst)
                else:
                    # V = X @ Wv (token-major)
                    brow = None
                    if b_dram is not None:
                        brow = b_pool.tile([1, D], F32R, tag="vb")
                        nc.sync.dma_start(brow[:], b_dram[:, :])
                    for t in range(T):
                        dst = vt_pool.tile([128, D], F32R, tag="t")
                        for c in range(2):
                            pp = ppj_pool.tile([128, 512], F32, tag="ppj")
                            for k in range(T):
                                nc.tensor.matmul(
                                    pp[:],
                                    xt[k][:, t * 128 : (t + 1) * 128],
                                    wt[k][:, c * 512 : (c + 1) * 512],
                                    start=(k == 0),
                                    stop=(k == T - 1) if brow is None else False,
                                )
                            if brow is not None:
                                nc.tensor.matmul(
                                    pp[:],
                                    ones_r[:],
                                    brow[:, c * 512 : (c + 1) * 512],
                                    start=False,
                                    stop=True,
                                )
                            nc.any.tensor_copy(
                                dst[:, c * 512 : (c + 1) * 512], pp[:]
                            )
                        VT.append(dst)

        # ---------------- Phase A: attention ----------------
        # opened after phase-P pools close so it reuses their SBUF space
        ot_pool = ctx.enter_context(tc.tile_pool(name="ot", bufs=T))
        OT = [ot_pool.tile([128, S], F32R, tag="t", name=f"ot_{j}") for j in range(T)]
        with (
            tc.tile_pool(name="rbp", bufs=3) as rb_pool,
            tc.tile_pool(name="ep", bufs=2) as e_pool,
            tc.tile_pool(name="ap", bufs=5) as a_pool,
            tc.tile_pool(name="etp", bufs=10) as et_pool,
            tc.tile_pool(name="mb", bufs=2) as mb_pool,
            tc.tile_pool(name="sm", bufs=4) as sm_pool,
            tc.tile_pool(name="psc", bufs=2, space="PSUM") as psc_pool,
            tc.tile_pool(name="pet", bufs=2, space="PSUM") as pet_pool,
            tc.tile_pool(name="pav", bufs=2, space="PSUM") as pav_pool,
        ):
            mbias = None
            if mask is not None:
                mbias = []
                for t in range(T):
                    mt = mb_pool.tile([128, S], I32, tag="mi")
                    nc.sync.dma_start(mt[:], mask[t * 128 : (t + 1) * 128, :])
                    mb = mb_pool.tile([128, S], F32R, tag="mf")
                    # (mask == 0) * -1e30
                    nc.vector.tensor_scalar(
                        mb[:], mt[:], 0, -1.0e30, op0=ALU.is_equal, op1=ALU.mult
                    )
                    mbias.append(mb)

            for h in range(H):
                ht, hr = h // 2, (h % 2) * 64
                a_tiles = []
                for t in range(T):
                    rbt = rb_pool.tile([128, S], F32R, tag="rb")
                    nc.sync.dma_start(rbt[:], rb[h, t * 128 : (t + 1) * 128, :])

                    ps = psc_pool.tile([128, S], F32, tag="sc")
                    for c in range(2):
                        sl = slice(c * 512, (c + 1) * 512)
                        nc.tensor.matmul(
                            ps[:, sl],
                            QT[ht][hr : hr + 64, t * 128 : (t + 1) * 128],
                            KT[ht][hr : hr + 64, sl],
                            start=True,
                            stop=False,
                        )
                        last = mbias is None
                        nc.tensor.matmul(
                            ps[:, sl],
                            ident_r[:],
                            rbt[:, sl],
                            start=False,
                            stop=last,
                        )
                        if mbias is not None:
                            nc.tensor.matmul(
                                ps[:, sl],
                                ident_r[:],
                                mbias[t][:, sl],
                                start=False,
                                stop=True,
                            )

                    et = e_pool.tile([128, S], F32, tag="e")
                    dnm = sm_pool.tile([128, 1], F32, tag="dnm")
                    nc.scalar.activation(et[:], ps[:], AF.Exp, accum_out=dnm[:])
                    inv = sm_pool.tile([128, 1], F32, tag="inv")
                    nc.vector.reciprocal(inv[:], dnm[:])
                    at = a_pool.tile([128, S], F32, tag="a")
                    nc.vector.tensor_scalar_mul(at[:], et[:], inv[:])
                    nc.sync.dma_start(attn[h, t * 128 : (t + 1) * 128, :], at[:])
                    a_tiles.append(at)

                    if t % 4 == 3:
                        qc = t // 4
                        pav = pav_pool.tile([64, 512], F32, tag="av")
                        for kb in range(T):
                            pe = pet_pool.tile([128, 512], F32, tag="pe")
                            for i in range(4):
                                nc.tensor.transpose(
                                    pe[:, i * 128 : (i + 1) * 128],
                                    a_tiles[qc * 4 + i][
                                        :, kb * 128 : (kb + 1) * 128
                                    ],
                                    ident[:],
                                )
                            etr = et_pool.tile([128, 512], F32R, tag="et")
                            nc.any.tensor_copy(etr[:], pe[:])
                            nc.tensor.matmul(
                                pav[:],
                                VT[kb][:, h * 64 : (h + 1) * 64],
                                etr[:],
                                start=(kb == 0),
                                stop=(kb == T - 1),
                            )
                        nc.any.tensor_copy(
                            OT[ht][hr : hr + 64, qc * 512 : (qc + 1) * 512], pav[:]
                        )

        # ---------------- Phase F: output projection ----------------
        with (
            tc.tile_pool(name="wo", bufs=T) as wo_pool,
            tc.tile_pool(name="ob", bufs=3) as o_pool,
            tc.tile_pool(name="pf", bufs=4, space="PSUM") as pf_pool,
        ):
            wt = []
            for k in range(T):
                wtile = wo_pool.tile([128, D], F32R, tag="w")
                nc.sync.dma_start(wtile[:], wo[k * 128 : (k + 1) * 128, :])
                wt.append(wtile)
            borow = None
            if bot is not None:
                borow = o_pool.tile([1, D], F32R, tag="bo")
                nc.sync.dma_start(borow[:], bot[:, :])
            for t in range(T):
                for c in range(2):
                    pf = pf_pool.tile([128, 512], F32, tag="pf")
                    for k in range(T):
                        nc.tensor.matmul(
                            pf[:],
                            OT[k][:, t * 128 : (t + 1) * 128],
                            wt[k][:, c * 512 : (c + 1) * 512],
                            start=(k == 0),
                            stop=(k == T - 1) if borow is None else False,
                        )
                    if borow is not None:
                        nc.tensor.matmul(
                            pf[:],
                            ones_r[:],
                            borow[:, c * 512 : (c + 1) * 512],
                            start=False,
                            stop=True,
                        )
                    ob = o_pool.tile([128, 512], F32, tag="ob")
                    nc.any.tensor_copy(ob[:], pf[:])
                    nc.sync.dma_start(
                        out[t * 128 : (t + 1) * 128, c * 512 : (c + 1) * 512], ob[:]
                    )


_NC_CACHE = {}


def _get_nc(flags):
    if flags not in _NC_CACHE:
        _NC_CACHE[flags] = build_nc(*flags)
    return _NC_CACHE[flags]


def kernel(
    query, key, value, attn_mask, rel_bias, Wq, bq, Wk, bk, Wv, bv, Wo, bo
):
    from concourse.bass_utils import run_bass_kernel_spmd

    query = np.ascontiguousarray(np.asarray(query, np.float32))
    key = np.ascontiguousarray(np.asarray(key, np.float32))
    value = np.ascontiguousarray(np.asarray(value, np.float32))
    attn_mask = np.asarray(attn_mask)
    rel_bias = np.ascontiguousarray(np.asarray(rel_bias, np.float32))
    Wq, Wk, Wv, Wo = (
        np.ascontiguousarray(np.asarray(w, np.float32)) for w in (Wq, Wk, Wv, Wo)
    )
    bq, bk, bv, bo = (np.asarray(b, np.float32) for b in (bq, bk, bv, bo))

    handle_mask = bool(np.any(attn_mask == 0))
    flags = (
        handle_mask,
        bool(np.any(bq)),
        bool(np.any(bk)),
        bool(np.any(bv)),
        bool(np.any(bo)),
    )
    nc = _get_nc(flags)

    in_maps = []
    for b in range(B):
        m = {
            "xq": query[b],
            "xk": key[b],
            "xv": value[b],
            "wq": Wq,
            "wk": Wk,
            "wv": Wv,
            "wo": Wo,
            "rb": rel_bias,
        }
        if flags[0]:
            m["mask"] = np.ascontiguousarray(attn_mask[b, 0].astype(np.int32))
        if flags[1]:
            m["bq"] = np.ascontiguousarray(bq.reshape(T, 128))
        if flags[2]:
            m["bk"] = np.ascontiguousarray(bk.reshape(T, 128))
        if flags[3]:
            m["bv"] = np.ascontiguousarray(bv.reshape(1, D))
        if flags[4]:
            m["bo"] = np.ascontiguousarray(bo.reshape(1, D))
        in_maps.append(m)

    res = run_bass_kernel_spmd(nc, in_maps, core_ids=list(range(NCORES)))
    out = np.stack([res.results[b]["out"] for b in range(B)])
    attn = np.stack([res.results[b]["attn"] for b in range(B)])
    return out, attn
